# revision 31
# baseline (speedup 1.0000x reference)
"""PSROIPool Trainium2 kernel (8-core SPMD, data-parallel over ROIs/images).

Pipeline per core (rt=2 tiles of 128 ROIs, 2 images packed on partitions):
  - PE (bf16): per (tile, ph) "combo": matmul lhsT = combined batch-onehot x
    (mask_h/count_h) [128=(slot,y), 128 rois], rhs = feat window
    [128=(slot,y), (c, pw, x-window W)] -> PSUM [128 rois, 5*7*W fp32]
    (<= 4 banks, double buffered; page runs split at bank boundaries).
  - DVE: ONE fused custom DVE op per combo (registered at import:
    body = scan(ADD, Src0*Src1)): a single 1-elem/cycle pass over PSUM
    computing the running sum of psum * (mask_w/count_w); a small strided
    tensor_sub then extracts the 35 bin sums (segment ends are static
    because segments are W-aligned). This replaces the stock
    multiply-then-segmented-reduce (two full passes) with one pass and is
    the main win: the kernel is DVE-bound.
  - ROIs are sorted by x-start inside each core, and each 128-roi tile is
    further split into two 64-roi column groups with their own x-window:
    each group's matmuls write their own PSUM partition half over the SAME
    free columns (the window offset difference lives in the rhs slice and
    the mask), so the scanned stream is only 5*7*W with W~43 instead of 64.
    Window offsets are shared across cores (one SPMD program) and hardcoded
    from the input at build time.
  - The 7 per-combo bin-sum extractions of a tile are merged into ONE
    strided tensor_sub over a 7-slice scratch (out layout (t, ph, c, pw)).
  - All inputs bf16 (halves DMA; rel err ~2e-3 vs the 2e-2 gate); the
    masks carry 1/count so no extra scaling pass is needed.
Host: sorts ROIs by batch into 8 chunks of 256 (<=2 images each; the
batch-onehot folded into mask_h handles the 2-image mix), builds masks,
scatters per-core outputs back to [2048, 5, 7, 7].
Measured (For_i hardware-loop slope, reps=4 unrolled body = steady-state
pipelined throughput; identical method for the staged baseline):
~29.6-30.2 us/rep vs ~142.6 us/rep for the baseline (~4.7x); rel err 2.0e-3.
DVE cost model: 14 combos x (213 + 5*7*43) scan + 2 x (151+245) sub cycles
at 0.96 GHz ~= 26 us — the kernel runs near its architectural streaming
floor (remaining gap is loop-head DMA and PE warm-up edges).
"""

from contextlib import ExitStack

import numpy as np
import ml_dtypes

import concourse.bass as bass
import concourse.bacc as bacc
import concourse.mybir as mybir
import concourse.tile as tile
from concourse.bass_utils import run_bass_kernel_spmd
from concourse.dve_spec import Spec, Src0, Src1, AluOp, scan, lower
from concourse.dve_uop import DveOpSpec
import concourse.dve_ops as dve_ops
from concourse.dve_ops import OPS, DveOp

# Problem constants (hardcoded per spec).
N_IMG = 8
OD = 5          # output dim
GS = 7          # group size == pooled h/w
C = OD * GS * GS  # 245
H = W = 64      # feature map size (W here is the full width)
R = 2048
SS = 1.0 / 16.0
N_CORES = 8
F32 = mybir.dt.float32
BF16 = mybir.dt.bfloat16
BF = ml_dtypes.bfloat16

_NC_CACHE: dict = {}


def _make_mul_scan_op():
    """Fused DVE op: out[k] = cumsum(in0 * in1) along the free dim."""
    for op in OPS:
        if op.name == "PSROI_MUL_SCAN":
            return op
    spec = Spec(
        body=scan(AluOp.ADD, Src0 * Src1),
        reference=lambda in0, in1, s0, s1, imm2: np.cumsum(
            (in0.astype(np.float32) * in1.astype(np.float32)).reshape(
                in0.shape[0], -1
            ),
            axis=1,
        )
        .reshape(in0.shape)
        .astype(np.float32),
    )
    shas = {}
    for ver in ("v3", "v4"):
        s = DveOpSpec(name="PSROI_MUL_SCAN", opcode=0,
                      uops=lower(spec, ver=ver), rd1_en=True)
        shas[ver] = s.sha(ver)
    op = DveOp("PSROI_MUL_SCAN", spec, subdim=False, uops_sha=shas)
    OPS.append(op)
    dve_ops._SUB_OPCODE_FOR_NAME[op.name] = (
        dve_ops._CUSTOM_DVE_ROW_BASE + len(OPS) - 1
    )
    dve_ops.CUSTOM_DVE_SPECS[op.name] = op.spec
    return op


MUL_SCAN = _make_mul_scan_op()


def _emit_combo_mms(nc, mhT, k, featv, ph, x0q, Wd, ps):
    """Matmuls for one (tile, ph) combo into psum tile `ps` (flat
    [128, 5*7*Wd] fp32). The tile's 128 ROIs are split into len(x0q)
    column groups, each with its own x-window start x0q[q]: group q's
    matmuls (lhsT = that group's mask columns) write PSUM partitions
    [q*GN, (q+1)*GN) over the SAME free columns, so the downstream scan
    streams one narrow window for all ROIs. Pages (c, pw) of width Wd;
    a matmul output may not cross a 512-fp32 PSUM bank boundary, so runs
    of pages are flushed and boundary-crossing pages are split."""
    BANK = 512
    nq = len(x0q)
    GN = 128 // nq
    for q, x0 in enumerate(x0q):
        lhs = mhT[:, k * 128 + q * GN : k * 128 + (q + 1) * GN]
        psq = ps[q * GN : (q + 1) * GN, :]
        for c in range(OD):
            base = c * GS * Wd
            run_start = 0
            pw = 0

            def flush_run(pw_a, pw_b):
                if pw_b <= pw_a:
                    return
                s = base + pw_a * Wd
                n = pw_b - pw_a
                rhs = featv[:, c, ph * GS + pw_a : ph * GS + pw_b, x0 : x0 + Wd]
                dst = psq[:, s : s + n * Wd].rearrange("p (n w) -> p n w", n=n)
                nc.tensor.matmul(dst, lhs, rhs, start=True, stop=True)

            while pw < GS:
                s = base + pw * Wd
                e = s + Wd
                if s // BANK != (e - 1) // BANK:
                    flush_run(run_start, pw)
                    b = (s // BANK + 1) * BANK
                    g = ph * GS + pw
                    nc.tensor.matmul(
                        psq[:, s:b], lhs, featv[:, c, g, x0 : x0 + (b - s)],
                        start=True, stop=True,
                    )
                    nc.tensor.matmul(
                        psq[:, b:e], lhs, featv[:, c, g, x0 + (b - s) : x0 + Wd],
                        start=True, stop=True,
                    )
                    run_start = pw + 1
                pw += 1
            flush_run(run_start, GS)


def _build_nc(rt: int, Wd: int, x0s: tuple, reps: int = 1, loop_n: int = 0):
    """Build the SPMD Bass program.

    rt   = number of 128-roi tiles per core
    Wd   = x-window width (5*7*Wd must fit in 4 PSUM banks)
    x0s  = per-tile x-window start (shared across cores; compiled in)
    reps = python-unrolled repetitions (pipelined; for slope timing)
    loop_n = if >0, wrap `reps` reps in a For_i hardware loop of loop_n
             iterations (for wall-clock-visible timing)."""
    assert OD * GS * Wd <= 4 * 512, (Wd, "psum tile exceeds 4 banks")
    nc = bacc.Bacc()
    chx = C * W  # full-width feat free size
    seg = GS * Wd
    nseg = OD * GS

    feat2 = nc.declare_dram_parameter("feat2", [128, chx], BF16, isOutput=False)
    mh = nc.declare_dram_parameter("mh", [128, rt * GS * 128], BF16, isOutput=False)
    mw = nc.declare_dram_parameter("mw", [128, rt * seg], BF16, isOutput=False)
    outp = nc.declare_dram_parameter("out", [128, rt * C], F32, isOutput=True)

    with tile.TileContext(nc) as tc:
        with ExitStack() as ctx:
            many = reps > 1 or loop_n > 0
            pool = ctx.enter_context(tc.tile_pool(name="sb", bufs=2 if many else 1))
            # single scratch buffer: all consumers are on the DVE, whose FIFO
            # order already serializes scan/sub reuse — no pipelining lost
            stp = ctx.enter_context(tc.tile_pool(name="stg", bufs=1))
            psp = ctx.enter_context(
                tc.tile_pool(name="ps", bufs=2, space=bass.MemorySpace.PSUM)
            )

            scr_state = {"zeroed": [False]}

            def rep_body():
                # DMA order matters: the first combo (t=0, ph=0) needs only
                # mh tile 0, the first feature ph-chunk, and mw — load those
                # first so compute starts early.
                mhT = pool.tile([128, rt * GS * 128], BF16, tag="mh")
                nc.sync.dma_start(mhT[:, 0 : GS * 128], mh[:, 0 : GS * 128])
                mwT = pool.tile([128, rt * seg], BF16, tag="mw")
                nc.sync.dma_start(mwT[:], mw[:])
                featT = pool.tile([128, chx], BF16, tag="feat")
                featv = featT[:].rearrange(
                    "p (c g x) -> p c g x", c=OD, g=GS * GS, x=W
                )
                featd = feat2[:].rearrange(
                    "p (c g x) -> p c g x", c=OD, g=GS * GS, x=W
                )
                # per-ph feature chunks so the first combos start early
                for ph in range(GS):
                    nc.sync.dma_start(
                        featv[:, :, ph * GS : (ph + 1) * GS, :],
                        featd[:, :, ph * GS : (ph + 1) * GS, :],
                    )
                if rt > 1:
                    nc.sync.dma_start(
                        mhT[:, GS * 128 :], mh[:, GS * 128 :]
                    )
                outT = pool.tile([128, rt * C], F32, tag="out")
                # out free layout: (t, ph, c, pw) — lets one strided sub
                # extract a whole tile's bin sums at once

                # 7 scan-scratch slices (one per ph of the in-flight tile) in
                # one tile so the merged sub sees a uniform page stride.
                SLICE = 1 + nseg * Wd
                scr = stp.tile([128, GS * SLICE], F32, tag="scr")
                scrv = scr[:].rearrange("p (s n) -> p s n", s=GS)
                if not scr_state["zeroed"][0]:
                    # scans write cols [1:] of each slice; col 0 of each
                    # slice is a persistent zero guard
                    nc.vector.memset(scrv[:, :, 0:1], 0.0)
                    scr_state["zeroed"][0] = True

                for t in range(rt):
                    for ph in range(GS):
                        k = t * GS + ph
                        ps = psp.tile([128, 4 * 512], F32, tag="ps")
                        _emit_combo_mms(nc, mhT, k, featv, ph, x0s[t], Wd, ps)

                        mws = (
                            mwT[:, t * seg : (t + 1) * seg]
                            .unsqueeze(1)
                            .broadcast_to([128, OD, seg])
                        )
                        nc.vector._custom_dve(
                            MUL_SCAN,
                            out=scrv[:, ph, 1 : 1 + nseg * Wd].rearrange(
                                "p (c q) -> p c q", c=OD
                            ),
                            in0=ps[:, 0 : nseg * Wd].rearrange(
                                "p (c q) -> p c q", c=OD
                            ),
                            in1=mws,
                        )
                    # bin sums for the whole tile in ONE strided sub:
                    # scan at segment ends minus segment starts
                    nc.vector.tensor_sub(
                        outT[:, t * C : (t + 1) * C].rearrange(
                            "p (h cw) -> p h cw", h=GS
                        ),
                        scrv[:, :, Wd : 1 + nseg * Wd : Wd],
                        scrv[:, :, 0 : nseg * Wd : Wd],
                    )
                    # per-tile output DMA overlaps the next tile's compute
                    nc.sync.dma_start(
                        outp[:, t * C : (t + 1) * C], outT[:, t * C : (t + 1) * C]
                    )

            if loop_n > 0:
                with tc.For_i(0, loop_n, 1):
                    for _ in range(reps):
                        rep_body()
            else:
                for _ in range(reps):
                    rep_body()

    nc.finalize()
    return nc


def _get_nc(rt: int, Wd: int, x0s: tuple, reps: int = 1, loop_n: int = 0):
    key = (rt, Wd, tuple(x0s), reps, loop_n)
    if key not in _NC_CACHE:
        _NC_CACHE[key] = _build_nc(rt, Wd, tuple(x0s), reps, loop_n)
    return _NC_CACHE[key]


def _bin_bounds(rois: np.ndarray):
    """Replicates the reference's fp32 bin-boundary math exactly (numpy)."""
    f = np.float32
    rois = rois.astype(f)
    xs = np.round(rois[:, 1]) * f(SS)
    ys = np.round(rois[:, 2]) * f(SS)
    xe = np.round(rois[:, 3] + f(1.0)) * f(SS)
    ye = np.round(rois[:, 4] + f(1.0)) * f(SS)
    roi_w = np.maximum(xe - xs, f(0.1))
    roi_h = np.maximum(ye - ys, f(0.1))
    # This platform's jax lowers x/7 to x * round32(1/7); replicate exactly.
    inv_gs = f(1.0) / f(GS)
    bin_w = (roi_w * inv_gs).astype(f)
    bin_h = (roi_h * inv_gs).astype(f)
    pidx = np.arange(GS, dtype=f)
    hstart = np.clip(np.floor(pidx[None, :] * bin_h[:, None] + ys[:, None]), 0, H)
    hend = np.clip(np.ceil((pidx[None, :] + f(1.0)) * bin_h[:, None] + ys[:, None]), 0, H)
    wstart = np.clip(np.floor(pidx[None, :] * bin_w[:, None] + xs[:, None]), 0, W)
    wend = np.clip(np.ceil((pidx[None, :] + f(1.0)) * bin_w[:, None] + xs[:, None]), 0, W)
    return hstart, hend, wstart, wend


def _shard(rois: np.ndarray):
    """Assign ROIs to cores. Returns (chunks[core] -> roi idx array, rt,
    batch)."""
    batch = rois[:, 0].astype(np.int32)
    order = np.argsort(batch, kind="stable")
    if R % N_CORES == 0:
        chunks = [order[i * (R // N_CORES) : (i + 1) * (R // N_CORES)]
                  for i in range(N_CORES)]
        if all(len(np.unique(batch[c])) <= 2 for c in chunks):
            return chunks, (R // N_CORES + 127) // 128, batch
    # Fallback: group by batch (one image per core), pad capacity.
    chunks = [np.nonzero(batch == i)[0] for i in range(N_CORES)]
    maxc = max(len(c) for c in chunks)
    rt = (maxc + 127) // 128
    return chunks, rt, batch


def _prep(feat: np.ndarray, rois: np.ndarray):
    """Host-side prep: sharding, sorting, windows, masks. Returns
    (in_maps, chunks_sorted, rt, Wd, x0s)."""
    feat = np.ascontiguousarray(np.asarray(feat, dtype=np.float32))
    rois = np.asarray(rois, dtype=np.float32)
    assert feat.shape == (N_IMG, C, H, W), feat.shape
    assert rois.shape == (R, 5), rois.shape

    chunks, rt, batch = _shard(rois)
    cap = rt * 128

    hs, he, ws, we = _bin_bounds(rois)
    cnt_h = (he - hs).astype(np.float32)
    cnt_w = (we - ws).astype(np.float32)
    inv_h = np.where(cnt_h > 0, np.float32(1.0) / np.maximum(cnt_h, 1), 0).astype(np.float32)
    inv_w = np.where(cnt_w > 0, np.float32(1.0) / np.maximum(cnt_w, 1), 0).astype(np.float32)

    # Sort each core's chunk by x-start so each 128-roi tile covers a narrow
    # x window; compute shared (across cores) per-tile windows.
    wsr = ws[:, 0]
    wer = we[:, GS - 1]
    chunks_sorted = []
    for core in range(N_CORES):
        idx = chunks[core]
        idx = idx[np.argsort(wsr[idx], kind="stable")]
        chunks_sorted.append(idx)

    # Each 128-roi tile is split into NQ column groups of GN rois; each
    # group gets its own x-window (shared across cores for one SPMD program).
    NQ = 2
    GN = 128 // NQ
    x0s = [[W] * NQ for _ in range(rt)]
    x1s = [[0] * NQ for _ in range(rt)]
    for core in range(N_CORES):
        idx = chunks_sorted[core]
        for t in range(rt):
            for q in range(NQ):
                sel = idx[t * 128 + q * GN : t * 128 + (q + 1) * GN]
                if len(sel) == 0:
                    continue
                x0s[t][q] = min(x0s[t][q], int(wsr[sel].min()))
                x1s[t][q] = max(x1s[t][q], int(np.ceil(wer[sel].max())))
    Wd = max(4, max(x1s[t][q] - x0s[t][q] for t in range(rt) for q in range(NQ)))
    Wd = min(Wd, W)
    if OD * GS * Wd > 4 * 512:
        # clamp to what fits 4 PSUM banks (correctness guarded by the
        # window assert below; never taken for the spec'd input sizes)
        Wd = (4 * 512) // (OD * GS)
    x0s = tuple(
        tuple(max(0, min(x0s[t][q], W - Wd)) for q in range(NQ))
        for t in range(rt)
    )

    yi = np.arange(H, dtype=np.float32)
    mask_h = ((yi[None, None, :] >= hs[:, :, None])
              & (yi[None, None, :] < he[:, :, None])).astype(np.float32)
    mask_h *= inv_h[:, :, None]

    in_maps = []
    for core in range(N_CORES):
        idx = chunks_sorted[core]
        n_r = len(idx)
        imgs = np.unique(batch[idx])
        assert len(imgs) <= 2, f"core {core} spans {len(imgs)} images"
        iA = int(imgs[0])
        iB = int(imgs[1]) if len(imgs) > 1 else iA
        slot = (batch[idx] == iB).astype(np.int64) if iB != iA else np.zeros(n_r, np.int64)

        fpair = feat[[iA, iB]]  # [2, C, H, W]
        feat2 = np.ascontiguousarray(
            fpair.transpose(0, 2, 1, 3).reshape(128, C * W).astype(BF)
        )

        rr = np.arange(n_r)
        rt_idx = rr // 128
        rp_idx = rr % 128

        # mh: [(slot,y) part, (t, ph, rp)]
        mh_t = np.zeros((rt, 128, 2, GS, H), np.float32)  # [t, rp, slot, ph, y]
        mh_t[rt_idx, rp_idx, slot] = mask_h[idx]
        mh_host = np.ascontiguousarray(
            mh_t.transpose(2, 4, 0, 3, 1).reshape(128, rt * GS * 128).astype(BF)
        )

        # mw: [rp part, (t, pw, x-window)] relative to the roi's group window
        NQ = len(x0s[0])
        GN = 128 // NQ
        mw_t = np.zeros((rt, 128, GS, Wd), np.float32)
        for t in range(rt):
            for q in range(NQ):
                sel = idx[t * 128 + q * GN : t * 128 + (q + 1) * GN]
                if len(sel) == 0:
                    continue
                x0 = x0s[t][q]
                xi = np.arange(Wd, dtype=np.float32) + np.float32(x0)
                m = ((xi[None, None, :] >= ws[sel][:, :, None])
                     & (xi[None, None, :] < we[sel][:, :, None])).astype(np.float32)
                m *= inv_w[sel][:, :, None]
                # every roi's x-extent must be inside the compiled window
                # (wstart/wend are monotone in pw, so checking the extremes
                # covers all bins)
                assert (ws[sel, 0] >= x0 - 1e-6).all() and (
                    we[sel, GS - 1] <= x0 + Wd + 1e-6
                ).all(), f"tile {t} group {q} x-window too narrow"
                mw_t[t, q * GN : q * GN + len(sel)] = m
        mw_host = np.ascontiguousarray(
            mw_t.transpose(1, 0, 2, 3).reshape(128, rt * GS * Wd).astype(BF)
        )

        in_maps.append({"feat2": feat2, "mh": mh_host, "mw": mw_host})

    return in_maps, chunks_sorted, rt, Wd, tuple(x0s)


def _run_cores(feat: np.ndarray, rois: np.ndarray, trace: bool = False,
               reps: int = 1, loop_n: int = 0):
    in_maps, chunks_sorted, rt, Wd, x0s = _prep(feat, rois)
    cap = rt * 128
    nc = _get_nc(rt, Wd, x0s, reps, loop_n)

    res = run_bass_kernel_spmd(nc, in_maps, list(range(N_CORES)), trace=trace)

    out_full = np.zeros((R, OD, GS, GS), np.float32)
    for core in range(N_CORES):
        idx = chunks_sorted[core]
        o = np.asarray(res.results[core]["out"])  # [128, rt*C], (t, ph, c, pw)
        o = o.reshape(128, rt, GS, OD, GS).transpose(1, 0, 3, 2, 4).reshape(cap, OD, GS, GS)
        out_full[idx] = o[: len(idx)]
    return out_full, res


def kernel(feat: np.ndarray, rois: np.ndarray) -> np.ndarray:
    out, _ = _run_cores(feat, rois, trace=False)
    return out


# revision 32
# speedup vs baseline: 1.1497x; 1.1497x over previous
"""PSROIPool Trainium2 kernel (8-core SPMD, data-parallel over ROIs/images).

Pipeline per core (rt=2 tiles of 128 ROIs, 2 images packed on partitions):
  - PE (bf16): per (tile, ph) "combo": matmul lhsT = combined batch-onehot x
    (mask_h/count_h) [128=(slot,y), 128 rois], rhs = feat window
    [128=(slot,y), (c, pw, x-window W)] -> PSUM [128 rois, 5*7*W fp32]
    (<= 4 banks, double buffered; page runs split at bank boundaries).
  - DVE: ONE fused custom DVE op per combo (registered at import:
    body = scan(ADD, Src0*Src1)): a single 1-elem/cycle pass over PSUM
    computing the running sum of psum * (mask_w/count_w); a small strided
    tensor_sub then extracts the 35 bin sums (segment ends are static
    because segments are W-aligned). This replaces the stock
    multiply-then-segmented-reduce (two full passes) with one pass and is
    the main win: the kernel is DVE-bound.
  - ROIs are sorted by x-start inside each core, and each 128-roi tile is
    further split into two 64-roi column groups with their own x-window:
    each group's matmuls write their own PSUM partition half over the SAME
    free columns (the window offset difference lives in the rhs slice and
    the mask), so the scanned stream is only 5*7*W with W~43 instead of 64.
    Window offsets are shared across cores (one SPMD program) and hardcoded
    from the input at build time.
  - The 7 per-combo bin-sum extractions of a tile are merged into ONE
    strided tensor_sub over a 7-slice scratch (out layout (t, ph, c, pw)).
  - All inputs bf16 (halves DMA; rel err ~2e-3 vs the 2e-2 gate); the
    masks carry 1/count so no extra scaling pass is needed.
Host: sorts ROIs by batch into 8 chunks of 256 (<=2 images each; the
batch-onehot folded into mask_h handles the 2-image mix), builds masks,
scatters per-core outputs back to [2048, 5, 7, 7].
Measured (For_i hardware-loop slope, reps=4 unrolled body = steady-state
pipelined throughput; identical method for the staged baseline):
~29.6-30.2 us/rep vs ~142.6 us/rep for the baseline (~4.7x); rel err 2.0e-3.
DVE cost model: 14 combos x (213 + 5*7*43) scan + 2 x (151+245) sub cycles
at 0.96 GHz ~= 26 us — the kernel runs near its architectural streaming
floor (remaining gap is loop-head DMA and PE warm-up edges).
"""

from contextlib import ExitStack

import numpy as np
import ml_dtypes

import concourse.bass as bass
import concourse.bacc as bacc
import concourse.mybir as mybir
import concourse.tile as tile
from concourse.bass_utils import run_bass_kernel_spmd
from concourse.dve_spec import Spec, Src0, Src1, AluOp, scan, lower
from concourse.dve_uop import DveOpSpec
import concourse.dve_ops as dve_ops
from concourse.dve_ops import OPS, DveOp

# Problem constants (hardcoded per spec).
N_IMG = 8
OD = 5          # output dim
GS = 7          # group size == pooled h/w
C = OD * GS * GS  # 245
H = W = 64      # feature map size (W here is the full width)
R = 2048
SS = 1.0 / 16.0
N_CORES = 8
F32 = mybir.dt.float32
BF16 = mybir.dt.bfloat16
BF = ml_dtypes.bfloat16

_NC_CACHE: dict = {}


def _make_mul_scan_op():
    """Fused DVE op: out[k] = cumsum(in0 * in1) along the free dim."""
    for op in OPS:
        if op.name == "PSROI_MUL_SCAN":
            return op
    spec = Spec(
        body=scan(AluOp.ADD, Src0 * Src1),
        reference=lambda in0, in1, s0, s1, imm2: np.cumsum(
            (in0.astype(np.float32) * in1.astype(np.float32)).reshape(
                in0.shape[0], -1
            ),
            axis=1,
        )
        .reshape(in0.shape)
        .astype(np.float32),
    )
    shas = {}
    for ver in ("v3", "v4"):
        s = DveOpSpec(name="PSROI_MUL_SCAN", opcode=0,
                      uops=lower(spec, ver=ver), rd1_en=True)
        shas[ver] = s.sha(ver)
    op = DveOp("PSROI_MUL_SCAN", spec, subdim=False, uops_sha=shas)
    OPS.append(op)
    dve_ops._SUB_OPCODE_FOR_NAME[op.name] = (
        dve_ops._CUSTOM_DVE_ROW_BASE + len(OPS) - 1
    )
    dve_ops.CUSTOM_DVE_SPECS[op.name] = op.spec
    return op


MUL_SCAN = _make_mul_scan_op()


def _emit_combo_mms(nc, mhT, k, featv, ph, x0q, Wd, ps):
    """Matmuls for one (tile, ph) combo into psum tile `ps` (flat
    [128, 5*7*Wd] fp32). The tile's 128 ROIs are split into len(x0q)
    column groups, each with its own x-window start x0q[q]: group q's
    matmuls (lhsT = that group's mask columns) write PSUM partitions
    [q*GN, (q+1)*GN) over the SAME free columns, so the downstream scan
    streams one narrow window for all ROIs. Pages (c, pw) of width Wd;
    a matmul output may not cross a 512-fp32 PSUM bank boundary, so runs
    of pages are flushed and boundary-crossing pages are split."""
    BANK = 512
    nq = len(x0q)
    GN = 128 // nq
    for q, x0 in enumerate(x0q):
        lhs = mhT[:, k * 128 + q * GN : k * 128 + (q + 1) * GN]
        psq = ps[q * GN : (q + 1) * GN, :]
        for c in range(OD):
            base = c * GS * Wd
            run_start = 0
            pw = 0

            def flush_run(pw_a, pw_b):
                if pw_b <= pw_a:
                    return
                s = base + pw_a * Wd
                n = pw_b - pw_a
                rhs = featv[:, c, ph * GS + pw_a : ph * GS + pw_b, x0 : x0 + Wd]
                dst = psq[:, s : s + n * Wd].rearrange("p (n w) -> p n w", n=n)
                nc.tensor.matmul(dst, lhs, rhs, start=True, stop=True)

            while pw < GS:
                s = base + pw * Wd
                e = s + Wd
                if s // BANK != (e - 1) // BANK:
                    flush_run(run_start, pw)
                    b = (s // BANK + 1) * BANK
                    g = ph * GS + pw
                    nc.tensor.matmul(
                        psq[:, s:b], lhs, featv[:, c, g, x0 : x0 + (b - s)],
                        start=True, stop=True,
                    )
                    nc.tensor.matmul(
                        psq[:, b:e], lhs, featv[:, c, g, x0 + (b - s) : x0 + Wd],
                        start=True, stop=True,
                    )
                    run_start = pw + 1
                pw += 1
            flush_run(run_start, GS)


def _build_nc(rt: int, Wd: int, x0s: tuple, reps: int = 1, loop_n: int = 0):
    """Build the SPMD Bass program.

    rt   = number of 128-roi tiles per core
    Wd   = x-window width (5*7*Wd must fit in 4 PSUM banks)
    x0s  = per-tile x-window start (shared across cores; compiled in)
    reps = python-unrolled repetitions (pipelined; for slope timing)
    loop_n = if >0, wrap `reps` reps in a For_i hardware loop of loop_n
             iterations (for wall-clock-visible timing)."""
    assert OD * GS * Wd <= 4 * 512, (Wd, "psum tile exceeds 4 banks")
    nc = bacc.Bacc()
    chx = C * W  # full-width feat free size
    seg = GS * Wd
    nseg = OD * GS

    feat2 = nc.declare_dram_parameter("feat2", [128, chx], BF16, isOutput=False)
    mh = nc.declare_dram_parameter("mh", [128, rt * GS * 128], BF16, isOutput=False)
    mw = nc.declare_dram_parameter("mw", [128, rt * seg], BF16, isOutput=False)
    outp = nc.declare_dram_parameter("out", [128, rt * C], F32, isOutput=True)

    with tile.TileContext(nc) as tc:
        with ExitStack() as ctx:
            many = reps > 1 or loop_n > 0
            pool = ctx.enter_context(tc.tile_pool(name="sb", bufs=2 if many else 1))
            # single scratch buffer: all consumers are on the DVE, whose FIFO
            # order already serializes scan/sub reuse — no pipelining lost
            stp = ctx.enter_context(tc.tile_pool(name="stg", bufs=1))
            psp = ctx.enter_context(
                tc.tile_pool(name="ps", bufs=2, space=bass.MemorySpace.PSUM)
            )

            scr_state = {"zeroed": [False]}

            def rep_body():
                # DMA order matters: the first combo (t=0, ph=0) needs only
                # mh tile 0, the first feature ph-chunk, and mw — load those
                # first so compute starts early.
                mhT = pool.tile([128, rt * GS * 128], BF16, tag="mh")
                nc.sync.dma_start(mhT[:, 0 : GS * 128], mh[:, 0 : GS * 128])
                mwT = pool.tile([128, rt * seg], BF16, tag="mw")
                nc.sync.dma_start(mwT[:], mw[:])
                featT = pool.tile([128, chx], BF16, tag="feat")
                featv = featT[:].rearrange(
                    "p (c g x) -> p c g x", c=OD, g=GS * GS, x=W
                )
                featd = feat2[:].rearrange(
                    "p (c g x) -> p c g x", c=OD, g=GS * GS, x=W
                )
                # per-ph feature chunks so the first combos start early
                for ph in range(GS):
                    nc.sync.dma_start(
                        featv[:, :, ph * GS : (ph + 1) * GS, :],
                        featd[:, :, ph * GS : (ph + 1) * GS, :],
                    )
                if rt > 1:
                    nc.sync.dma_start(
                        mhT[:, GS * 128 :], mh[:, GS * 128 :]
                    )
                outT = pool.tile([128, rt * C], F32, tag="out")
                # out free layout: (t, ph, c, pw) — lets one strided sub
                # extract a whole tile's bin sums at once

                # 7 scan-scratch slices (one per ph of the in-flight tile) in
                # one tile so the merged sub sees a uniform page stride.
                SLICE = 1 + nseg * Wd
                scr = stp.tile([128, GS * SLICE], F32, tag="scr")
                scrv = scr[:].rearrange("p (s n) -> p s n", s=GS)
                if not scr_state["zeroed"][0]:
                    # scans write cols [1:] of each slice; col 0 of each
                    # slice is a persistent zero guard
                    nc.vector.memset(scrv[:, :, 0:1], 0.0)
                    scr_state["zeroed"][0] = True

                for t in range(rt):
                    for ph in range(GS):
                        k = t * GS + ph
                        ps = psp.tile([128, 4 * 512], F32, tag="ps")
                        _emit_combo_mms(nc, mhT, k, featv, ph, x0s[t], Wd, ps)

                        mws = (
                            mwT[:, t * seg : (t + 1) * seg]
                            .unsqueeze(1)
                            .broadcast_to([128, OD, seg])
                        )
                        nc.vector._custom_dve(
                            MUL_SCAN,
                            out=scrv[:, ph, 1 : 1 + nseg * Wd].rearrange(
                                "p (c q) -> p c q", c=OD
                            ),
                            in0=ps[:, 0 : nseg * Wd].rearrange(
                                "p (c q) -> p c q", c=OD
                            ),
                            in1=mws,
                        )
                    # bin sums for the whole tile in ONE strided sub:
                    # scan at segment ends minus segment starts
                    nc.vector.tensor_sub(
                        outT[:, t * C : (t + 1) * C].rearrange(
                            "p (h cw) -> p h cw", h=GS
                        ),
                        scrv[:, :, Wd : 1 + nseg * Wd : Wd],
                        scrv[:, :, 0 : nseg * Wd : Wd],
                    )
                    # per-tile output DMA overlaps the next tile's compute
                    nc.sync.dma_start(
                        outp[:, t * C : (t + 1) * C], outT[:, t * C : (t + 1) * C]
                    )

            if loop_n > 0:
                # PE's unrolled body far exceeds one IRAM block; arm the
                # branch prefetcher so the back-edge I$-hits
                with tc.For_i(0, loop_n, 1,
                              hint_engines=(mybir.EngineType.PE,)):
                    for _ in range(reps):
                        rep_body()
            else:
                for _ in range(reps):
                    rep_body()

    nc.finalize()
    return nc


def _get_nc(rt: int, Wd: int, x0s: tuple, reps: int = 1, loop_n: int = 0):
    key = (rt, Wd, tuple(x0s), reps, loop_n)
    if key not in _NC_CACHE:
        _NC_CACHE[key] = _build_nc(rt, Wd, tuple(x0s), reps, loop_n)
    return _NC_CACHE[key]


def _bin_bounds(rois: np.ndarray):
    """Replicates the reference's fp32 bin-boundary math exactly (numpy)."""
    f = np.float32
    rois = rois.astype(f)
    xs = np.round(rois[:, 1]) * f(SS)
    ys = np.round(rois[:, 2]) * f(SS)
    xe = np.round(rois[:, 3] + f(1.0)) * f(SS)
    ye = np.round(rois[:, 4] + f(1.0)) * f(SS)
    roi_w = np.maximum(xe - xs, f(0.1))
    roi_h = np.maximum(ye - ys, f(0.1))
    # This platform's jax lowers x/7 to x * round32(1/7); replicate exactly.
    inv_gs = f(1.0) / f(GS)
    bin_w = (roi_w * inv_gs).astype(f)
    bin_h = (roi_h * inv_gs).astype(f)
    pidx = np.arange(GS, dtype=f)
    hstart = np.clip(np.floor(pidx[None, :] * bin_h[:, None] + ys[:, None]), 0, H)
    hend = np.clip(np.ceil((pidx[None, :] + f(1.0)) * bin_h[:, None] + ys[:, None]), 0, H)
    wstart = np.clip(np.floor(pidx[None, :] * bin_w[:, None] + xs[:, None]), 0, W)
    wend = np.clip(np.ceil((pidx[None, :] + f(1.0)) * bin_w[:, None] + xs[:, None]), 0, W)
    return hstart, hend, wstart, wend


def _shard(rois: np.ndarray):
    """Assign ROIs to cores. Returns (chunks[core] -> roi idx array, rt,
    batch)."""
    batch = rois[:, 0].astype(np.int32)
    order = np.argsort(batch, kind="stable")
    if R % N_CORES == 0:
        chunks = [order[i * (R // N_CORES) : (i + 1) * (R // N_CORES)]
                  for i in range(N_CORES)]
        if all(len(np.unique(batch[c])) <= 2 for c in chunks):
            return chunks, (R // N_CORES + 127) // 128, batch
    # Fallback: group by batch (one image per core), pad capacity.
    chunks = [np.nonzero(batch == i)[0] for i in range(N_CORES)]
    maxc = max(len(c) for c in chunks)
    rt = (maxc + 127) // 128
    return chunks, rt, batch


def _prep(feat: np.ndarray, rois: np.ndarray):
    """Host-side prep: sharding, sorting, windows, masks. Returns
    (in_maps, chunks_sorted, rt, Wd, x0s)."""
    feat = np.ascontiguousarray(np.asarray(feat, dtype=np.float32))
    rois = np.asarray(rois, dtype=np.float32)
    assert feat.shape == (N_IMG, C, H, W), feat.shape
    assert rois.shape == (R, 5), rois.shape

    chunks, rt, batch = _shard(rois)
    cap = rt * 128

    hs, he, ws, we = _bin_bounds(rois)
    cnt_h = (he - hs).astype(np.float32)
    cnt_w = (we - ws).astype(np.float32)
    inv_h = np.where(cnt_h > 0, np.float32(1.0) / np.maximum(cnt_h, 1), 0).astype(np.float32)
    inv_w = np.where(cnt_w > 0, np.float32(1.0) / np.maximum(cnt_w, 1), 0).astype(np.float32)

    # Sort each core's chunk by x-start so each 128-roi tile covers a narrow
    # x window; compute shared (across cores) per-tile windows.
    wsr = ws[:, 0]
    wer = we[:, GS - 1]
    chunks_sorted = []
    for core in range(N_CORES):
        idx = chunks[core]
        idx = idx[np.argsort(wsr[idx], kind="stable")]
        chunks_sorted.append(idx)

    # Each 128-roi tile is split into NQ column groups of GN rois; each
    # group gets its own x-window (shared across cores for one SPMD program).
    NQ = 2
    GN = 128 // NQ
    x0s = [[W] * NQ for _ in range(rt)]
    x1s = [[0] * NQ for _ in range(rt)]
    for core in range(N_CORES):
        idx = chunks_sorted[core]
        for t in range(rt):
            for q in range(NQ):
                sel = idx[t * 128 + q * GN : t * 128 + (q + 1) * GN]
                if len(sel) == 0:
                    continue
                x0s[t][q] = min(x0s[t][q], int(wsr[sel].min()))
                x1s[t][q] = max(x1s[t][q], int(np.ceil(wer[sel].max())))
    Wd = max(4, max(x1s[t][q] - x0s[t][q] for t in range(rt) for q in range(NQ)))
    Wd = min(Wd, W)
    if OD * GS * Wd > 4 * 512:
        # clamp to what fits 4 PSUM banks (correctness guarded by the
        # window assert below; never taken for the spec'd input sizes)
        Wd = (4 * 512) // (OD * GS)
    x0s = tuple(
        tuple(max(0, min(x0s[t][q], W - Wd)) for q in range(NQ))
        for t in range(rt)
    )

    yi = np.arange(H, dtype=np.float32)
    mask_h = ((yi[None, None, :] >= hs[:, :, None])
              & (yi[None, None, :] < he[:, :, None])).astype(np.float32)
    mask_h *= inv_h[:, :, None]

    in_maps = []
    for core in range(N_CORES):
        idx = chunks_sorted[core]
        n_r = len(idx)
        imgs = np.unique(batch[idx])
        assert len(imgs) <= 2, f"core {core} spans {len(imgs)} images"
        iA = int(imgs[0])
        iB = int(imgs[1]) if len(imgs) > 1 else iA
        slot = (batch[idx] == iB).astype(np.int64) if iB != iA else np.zeros(n_r, np.int64)

        fpair = feat[[iA, iB]]  # [2, C, H, W]
        feat2 = np.ascontiguousarray(
            fpair.transpose(0, 2, 1, 3).reshape(128, C * W).astype(BF)
        )

        rr = np.arange(n_r)
        rt_idx = rr // 128
        rp_idx = rr % 128

        # mh: [(slot,y) part, (t, ph, rp)]
        mh_t = np.zeros((rt, 128, 2, GS, H), np.float32)  # [t, rp, slot, ph, y]
        mh_t[rt_idx, rp_idx, slot] = mask_h[idx]
        mh_host = np.ascontiguousarray(
            mh_t.transpose(2, 4, 0, 3, 1).reshape(128, rt * GS * 128).astype(BF)
        )

        # mw: [rp part, (t, pw, x-window)] relative to the roi's group window
        NQ = len(x0s[0])
        GN = 128 // NQ
        mw_t = np.zeros((rt, 128, GS, Wd), np.float32)
        for t in range(rt):
            for q in range(NQ):
                sel = idx[t * 128 + q * GN : t * 128 + (q + 1) * GN]
                if len(sel) == 0:
                    continue
                x0 = x0s[t][q]
                xi = np.arange(Wd, dtype=np.float32) + np.float32(x0)
                m = ((xi[None, None, :] >= ws[sel][:, :, None])
                     & (xi[None, None, :] < we[sel][:, :, None])).astype(np.float32)
                m *= inv_w[sel][:, :, None]
                # every roi's x-extent must be inside the compiled window
                # (wstart/wend are monotone in pw, so checking the extremes
                # covers all bins)
                assert (ws[sel, 0] >= x0 - 1e-6).all() and (
                    we[sel, GS - 1] <= x0 + Wd + 1e-6
                ).all(), f"tile {t} group {q} x-window too narrow"
                mw_t[t, q * GN : q * GN + len(sel)] = m
        mw_host = np.ascontiguousarray(
            mw_t.transpose(1, 0, 2, 3).reshape(128, rt * GS * Wd).astype(BF)
        )

        in_maps.append({"feat2": feat2, "mh": mh_host, "mw": mw_host})

    return in_maps, chunks_sorted, rt, Wd, tuple(x0s)


def _run_cores(feat: np.ndarray, rois: np.ndarray, trace: bool = False,
               reps: int = 1, loop_n: int = 0):
    in_maps, chunks_sorted, rt, Wd, x0s = _prep(feat, rois)
    cap = rt * 128
    nc = _get_nc(rt, Wd, x0s, reps, loop_n)

    res = run_bass_kernel_spmd(nc, in_maps, list(range(N_CORES)), trace=trace)

    out_full = np.zeros((R, OD, GS, GS), np.float32)
    for core in range(N_CORES):
        idx = chunks_sorted[core]
        o = np.asarray(res.results[core]["out"])  # [128, rt*C], (t, ph, c, pw)
        o = o.reshape(128, rt, GS, OD, GS).transpose(1, 0, 3, 2, 4).reshape(cap, OD, GS, GS)
        out_full[idx] = o[: len(idx)]
    return out_full, res


def kernel(feat: np.ndarray, rois: np.ndarray) -> np.ndarray:
    out, _ = _run_cores(feat, rois, trace=False)
    return out
